# revision 15
# baseline (speedup 1.0000x reference)
"""Additive (Bahdanau) attention on 8 TRN2 NeuronCores — self-contained Bass kernel.

Math: score(q,k) = w2 . tanh(hq[q] + hk[k] + b1) + b2;  out = softmax_k(score) @ V.

tanh(s) ~= sum_m c_m sin(w_m s) with a 3-term DOUBLING basis w_m = {1,2,4}*W0,
W0=0.6 (weighted-LSQ fit; e2e rel-err ~1.0e-2, bf16).  ScalarE's Sin spline is
accurate to ~2e-3 out to |x|~3.9, so the pi/2-shifted cos args stay in range.
Angle addition sin(w(a+b)) = sin(wa)cos(wb)+cos(wa)sin(wb) turns the [B,Q,K,D]
tanh+reduce into 48 TensorE matmuls contracting over D.

Per side (F=queries, G=keys), features per level [scale]:
  m0: s1=sin(W0 h) [1],  c1=sin(W0 h + pi/2) [1]       (ScalarE, from PSUM)
  m1: S2=s1*c1 [1/2],    C2=0.5-s1^2 [1/2]             (DVE bf16 products)
  m2: S4=S2*C2 [1/8],    C4=0.125-S2^2 [1/8]
b1 folds into the G-side Sin biases; w2 and c_m fold into per-partition scales
of the F-side features; the m2 sin-scaled feature is fused via
scalar_tensor_tensor and the m2 cos-scaled feature comes from an Identity
activation with AP bias+scale on S2F^2 (no unscaled C4F/S4F materialized).

Softmax: b2 drops (shift invariance); the denominator comes from tiny ones-
column matmuls reusing the attn weights; reciprocal on VectorE folds into
per-chunk output scales (split across ScalarE/VectorE).

Hardware lessons baked in:
- q/k arrive PRE-TRANSPOSED via the DMA transpose XBAR (no PE transposes, no
  PSUM copies, no identity matrix).  Any DMA sem-waits earlier-pushed
  in-flight transposes and vice versa, and two transposes serialize in the
  XBAR — so qT's push is the FIRST DMA anywhere, kT follows, small inputs
  ride the software-DGE queue, and v's push is gated on qT's completion.
- PSUM "start" clears the WHOLE bank's has_written bits: every interleaved
  accumulation group owns its own bank (4 bank-padded logits accumulators),
  and groups sharing a bank (attn@V chunks, denominators) are fully
  serialized.  h_f/h_g banks are reused as den/avt storage.
- ScalarE activations misread strided sources: all ACT sources are
  contiguous slices.
- A wide dummy-matmul warm-up burst holds the PE HAM clock gate at 2.4GHz
  through the DMA phase; a dummy Sin hoists the trig ACT-table load to
  program start; a dummy Exp (gated on the last G sin) hides the exp-table
  switch behind the logits matmuls.

Sharding: data-parallel over batch, B=16 -> 2 per core, no collectives.
"""

import math
from contextlib import ExitStack

import numpy as np
import ml_dtypes

import concourse.bass as bass
import concourse.mybir as mybir
import concourse.tile as tile
from concourse import bacc
from concourse.bass_utils import run_bass_kernel_spmd

F32 = mybir.dt.float32
BF16 = mybir.dt.bfloat16
AF = mybir.ActivationFunctionType
ALU = mybir.AluOpType

NCORES = 8
B, NQ, NK, D = 16, 256, 256, 256
BL = B // NCORES          # local batches per core = 2
P = 128
DC = D // P               # d-chunks = 2
EC = D // P               # e-chunks (contraction for hq/hk matmuls) = 2
QT = NQ // P              # q-tiles = 2
KT = NK // P              # k-tiles = 2
M_SINES = 3
W = BL * NQ               # 512: (b, q) free width per dt slice
DV1 = D + 1               # values + ones column
NWARM = 12                # wide PE warm-up matmuls (N=512, ~0.43us each cold)

# {1,2,4}*W0 weighted-LSQ fit of tanh (Gaussian(1.0)+5e-4 weight)
W0 = 0.6
COEF = (1.02555, 0.13778, 0.08549)
MCOEF = (COEF[0], 4.0 * COEF[1], 64.0 * COEF[2])   # 1/alpha2 folded in

# tbl columns
TB_HPI = 0                 # pi/2
TB_WB1 = 1                 # [dt] W0*b1
TB_WB1H = 1 + DC           # [dt] W0*b1 + pi/2
TB_W2C = 1 + 2 * DC        # [m*DC+dt] MCOEF[m]*w2
TB_NW2C2 = TB_W2C + M_SINES * DC   # [dt] -MCOEF[2]*w2
TB_W2C28 = TB_NW2C2 + DC           # [dt] 0.125*MCOEF[2]*w2
TB_N = TB_W2C28 + DC


def build_kernel() -> bacc.Bacc:
    nc = bacc.Bacc("TRN2", target_bir_lowering=False, debug=False)

    qk_d = nc.dram_tensor("qk", [2 * BL * NQ, D], BF16, kind="ExternalInput").ap()
    v_d = nc.dram_tensor("values", [BL, NK, D], BF16, kind="ExternalInput").ap()
    wq_d = nc.dram_tensor("wq", [P, EC * DC * P], BF16, kind="ExternalInput").ap()
    wk_d = nc.dram_tensor("wk", [P, EC * DC * P], BF16, kind="ExternalInput").ap()
    tbl_d = nc.dram_tensor("tbl", [P, TB_N], F32, kind="ExternalInput").ap()
    out_d = nc.dram_tensor("out", [BL, NQ, D], F32, kind="ExternalOutput").ap()

    with tile.TileContext(nc) as tc, ExitStack() as ctx:
        cpool = ctx.enter_context(tc.tile_pool(name="consts", bufs=1))
        dpool = ctx.enter_context(tc.tile_pool(name="data", bufs=1))

        dummy = cpool.tile([P, 648], BF16)
        wq = cpool.tile([P, EC * DC * P], BF16)
        wk = cpool.tile([P, EC * DC * P], BF16)
        tbl = cpool.tile([P, TB_N], F32)
        qTt = dpool.tile([P, EC * W], BF16)     # col = (ec, b, q)
        kTt = dpool.tile([P, EC * W], BF16)
        vb = dpool.tile([P, BL * KT * D], BF16)
        ones1 = dpool.tile([P, 1], BF16)

        # memsets first so the dummy Sin (ACT-table hoist) can run immediately
        nc.vector.memset(dummy[:], 0.0)
        nc.vector.memset(ones1[:], 1.0)

        # ---- input DMAs ----
        # Empirically-best arrangement: qT's push is the FIRST DMA anywhere
        # (any DMA sem-waits earlier-pushed in-flight transposes and vice
        # versa, and the two transposes serialize in the XBAR).  tbl rides
        # the software queue concurrently; weights follow their queue's
        # transpose; v is gated on qT's completion so its software transfer
        # does not compete with kT's transfer window.
        nc.sync.dma_start(
            qTt[:].rearrange("p (ec w) -> p ec w", ec=EC),
            qk_d[0:BL * NQ], transpose=True)
        nc.gpsimd.dma_start(tbl[:], tbl_d[:])
        nc.scalar.dma_start(
            kTt[:].rearrange("p (ec w) -> p ec w", ec=EC),
            qk_d[BL * NQ:2 * BL * NQ], transpose=True)
        nc.scalar.activation(dummy[:, 641:642], dummy[:, 640:641], AF.Sin)
        nc.sync.dma_start(wq[:], wq_d[:])
        nc.scalar.dma_start(wk[:], wk_d[:])
        nc.gpsimd.tensor_copy(dummy[:, 644:645], qTt[:, 0:1])   # v waits qT
        nc.gpsimd.dma_start(
            vb[:].rearrange("p (b t e) -> p b t e", b=BL, t=KT),
            v_d.rearrange("b (t p) e -> p b t e", p=P))

        halfpi = tbl[:, TB_HPI:TB_HPI + 1]

        def tcol(c):
            return tbl[:, c:c + 1]

        # feature tiles: col = (dt, half, b, q); half 0 = sin-side, 1 = cos-side
        f1 = dpool.tile([P, DC * 2 * W], BF16)
        g1 = dpool.tile([P, DC * 2 * W], BF16)
        f2 = dpool.tile([P, DC * 2 * W], BF16)
        g2 = dpool.tile([P, DC * 2 * W], BF16)
        g4 = dpool.tile([P, DC * 2 * W], BF16)
        p1f = dpool.tile([P, DC * W], BF16)
        p1g = dpool.tile([P, DC * W], BF16)
        p2f = dpool.tile([P, DC * W], BF16)
        p2g = dpool.tile([P, DC * W], BF16)
        sFs = [dpool.tile([P, DC * 2 * W], BF16, name=f"sF{m}") for m in range(M_SINES)]
        expT = dpool.tile([P, KT * W], BF16)    # col = (kt, b, q)

        # single flat PSUM pool — no pool close/reuse machinery:
        #   h_f (2 banks, later reused as the two logits accumulators),
        #   h_g (2 banks, also the warm-up dst), av (2), den (1)
        ppool = ctx.enter_context(tc.tile_pool(name="psum", bufs=1, space="PSUM"))
        h_f = ppool.tile([P, DC * W], F32, name="h_f")
        h_g = ppool.tile([P, DC * W], F32, name="h_g")

        # ---- PE warm-up burst (HAM clock): wide dummy matmuls while DMAs land
        for _ in range(NWARM):
            nc.tensor.matmul(h_g[:, 0:512], dummy[:, 0:P], dummy[:, P:P + 512],
                             start=True, stop=True)

        if True:
            # hq then hk: N=512 matmuls (b merged), accumulate over ec;
            # dummy bursts between phases keep the HAM clock gate warm while
            # kT lands and while the Sin chain runs
            for side, h in ((0, h_f), (1, h_g)):
                wgt = (wq, wk)[side]
                src_t = (qTt, kTt)[side]
                if side == 1:
                    for _ in range(8):
                        nc.tensor.matmul(h_g[:, 0:512], dummy[:, 0:P],
                                         dummy[:, P:P + 512], start=True, stop=True)
                for dt in range(DC):
                    for ec in range(EC):
                        nc.tensor.matmul(
                            h[:, dt * W:(dt + 1) * W],
                            wgt[:, (ec * DC + dt) * P:(ec * DC + dt + 1) * P],
                            src_t[:, ec * W:(ec + 1) * W],
                            start=(ec == 0), stop=(ec == EC - 1))

            # ---- base trig on ScalarE (PSUM-source), contiguous slices ----
            def fsl(t, dt, half):
                return t[:, (dt * 2 + half) * W:(dt * 2 + half + 1) * W]

            for dt in range(DC):
                nc.scalar.activation(fsl(f1, dt, 0), h_f[:, dt * W:(dt + 1) * W],
                                     AF.Sin, bias=0.0, scale=W0)
            for dt in range(DC):
                nc.scalar.activation(fsl(f1, dt, 1), h_f[:, dt * W:(dt + 1) * W],
                                     AF.Sin, bias=halfpi, scale=W0)
            for dt in range(DC):   # cos-G first: m0/pi0 weights
                nc.scalar.activation(
                    fsl(g1, dt, 1), h_g[:, dt * W:(dt + 1) * W],
                    AF.Sin, bias=tcol(TB_WB1H + dt), scale=W0)
            for dt in range(DC):
                nc.scalar.activation(
                    fsl(g1, dt, 0), h_g[:, dt * W:(dt + 1) * W],
                    AF.Sin, bias=tcol(TB_WB1 + dt), scale=W0)

        # keep HAM busy between hk and the first logits matmuls (h_f is
        # consumed by the F sins above; den's start-matmuls clear it later)
        for _ in range(8):
            nc.tensor.matmul(h_f[:, 0:512], dummy[:, 0:P], dummy[:, P:P + 512],
                             start=True, stop=True)

        # ---- feature chain: source order ~ execution order ----
        def fsl2(t, dt, half):
            return t[:, (dt * 2 + half) * W:(dt * 2 + half + 1) * W]

        def dsl(t, dt):
            return t[:, dt * W:(dt + 1) * W]

        for dt in range(DC):       # p1f = s1F^2 (Vector)
            nc.vector.tensor_tensor(dsl(p1f, dt), fsl2(f1, dt, 0), fsl2(f1, dt, 0),
                                    op=ALU.mult)
        for dt in range(DC):       # m0 scaled F features (both halves, 1024)
            nc.vector.tensor_scalar_mul(
                sFs[0][:, dt * 2 * W:(dt + 1) * 2 * W],
                f1[:, dt * 2 * W:(dt + 1) * 2 * W], tcol(TB_W2C + 0 * DC + dt))
        for dt in range(DC):       # S2F, C2F
            nc.vector.tensor_tensor(fsl2(f2, dt, 0), fsl2(f1, dt, 0), fsl2(f1, dt, 1),
                                    op=ALU.mult)
            nc.vector.tensor_scalar(fsl2(f2, dt, 1), dsl(p1f, dt), -1.0, 0.5,
                                    op0=ALU.mult, op1=ALU.add)
        for dt in range(DC):       # m1 scaled F features
            nc.vector.tensor_scalar_mul(
                sFs[1][:, dt * 2 * W:(dt + 1) * 2 * W],
                f2[:, dt * 2 * W:(dt + 1) * 2 * W], tcol(TB_W2C + 1 * DC + dt))
        for dt in range(DC):       # m2 sin-scaled F via STT (only needs f2)
            nc.vector.scalar_tensor_tensor(
                fsl2(sFs[2], dt, 0), fsl2(f2, dt, 0),
                tcol(TB_W2C + 2 * DC + dt), fsl2(f2, dt, 1),
                op0=ALU.mult, op1=ALU.mult)
        for dt in range(DC):       # G chain start
            nc.vector.tensor_tensor(dsl(p1g, dt), fsl2(g1, dt, 0), fsl2(g1, dt, 0),
                                    op=ALU.mult)
            nc.vector.tensor_tensor(fsl2(g2, dt, 0), fsl2(g1, dt, 0), fsl2(g1, dt, 1),
                                    op=ALU.mult)
            nc.vector.tensor_scalar(fsl2(g2, dt, 1), dsl(p1g, dt), -1.0, 0.5,
                                    op0=ALU.mult, op1=ALU.add)
        # p2f squares and scaled C4F on Vector (keeps ScalarE's queue clean
        # for the sins: a Scalar detour here made the scheduler thrash the
        # ACT tables and delay the last G sin by ~3.5us); then exp-table hoist
        c4f = dpool.tile([P, DC * W], BF16)
        for dt in range(DC):
            nc.vector.tensor_tensor(dsl(p2f, dt), fsl2(f2, dt, 0), fsl2(f2, dt, 0),
                                    op=ALU.mult)
        for dt in range(DC):
            nc.vector.tensor_scalar(dsl(c4f, dt), dsl(p2f, dt), -1.0, 0.125,
                                    op0=ALU.mult, op1=ALU.add)
            nc.vector.tensor_scalar_mul(
                sFs[2][:, (dt * 2 + 1) * W:(dt * 2 + 2) * W], dsl(c4f, dt),
                tcol(TB_W2C + 2 * DC + dt))
        # gate cell = last col of the sin-G dt1 half — written by the LAST
        # Sin in queue order, so the exp-table switch cannot preempt any sin
        nc.scalar.activation(dummy[:, 642:643], g1[:, 3 * W - 1:3 * W],
                             AF.Exp)
        # G chain rest
        for dt in range(DC):
            nc.vector.tensor_tensor(dsl(p2g, dt), fsl2(g2, dt, 0), fsl2(g2, dt, 0),
                                    op=ALU.mult)
            nc.vector.tensor_tensor(fsl2(g4, dt, 0), fsl2(g2, dt, 0), fsl2(g2, dt, 1),
                                    op=ALU.mult)
            nc.vector.tensor_scalar(fsl2(g4, dt, 1), dsl(p2g, dt), -1.0, 0.125,
                                    op0=ALU.mult, op1=ALU.add)

        # ---- logits matmuls: 48 N=256 mms, m-major ----
        # PSUM "start" clears the WHOLE bank's has_written bits, so every
        # interleaved accumulation group needs its own bank: one bank-padded
        # [P,256] accumulator per (kt, b).
        lg_ps = [[ppool.tile([P, NQ], F32, padded_shape=[P, 512],
                             name=f"lg{kt}{b}") for b in range(BL)]
                 for kt in range(KT)]

        def lg_mm(gt, half, m, dt, b, kt, start, stop):
            nc.tensor.matmul(
                lg_ps[kt][b][:],
                gt[:, (dt * 2 + half) * W + b * NQ + kt * P:
                   (dt * 2 + half) * W + b * NQ + (kt + 1) * P],
                sFs[m][:, (dt * 2 + (1 - half)) * W + b * NQ:
                       (dt * 2 + (1 - half)) * W + (b + 1) * NQ],
                start=start, stop=stop)

        # m0: pi0 (cos-G x sin-F) then pi1; m1 same
        for m, gt in ((0, g1), (1, g2)):
            for half in (1, 0):
                for dt in range(DC):
                    for b in range(BL):
                        for kt in range(KT):
                            lg_mm(gt, half, m, dt, b, kt,
                                  m == 0 and half == 1 and dt == 0, False)
        # m2: kt-major, cos-G x sin-F (STT, ready first) then sin-G x cos-F;
        # exp per (kt, b) as soon as its accumulator stops
        for kt in range(KT):
            for half in (1, 0):
                for dt in range(DC):
                    for b in range(BL):
                        lg_mm(g4, half, 2, dt, b, kt, False,
                              half == 0 and dt == DC - 1)
            for b in range(BL):
                nc.scalar.activation(
                    expT[:, kt * W + b * NQ:kt * W + (b + 1) * NQ],
                    lg_ps[kt][b][:], AF.Exp)

        # ---- attn @ V, denominator via ones-column matmuls on the same
        # weights.  Groups sharing a bank are fully serialized (chunk-major,
        # kt innermost) per the bank-wide start-clear semantics; avt/den
        # reuse the h_g/h_f banks (their readers are long done).
        rcols = [cpool.tile([P, 1], F32, name=f"rc{i}") for i in range(BL * QT)]
        avt = h_g
        den = h_f
        out_sbs = [dpool.tile([P, D], F32, name=f"osb{i}") for i in range(BL * QT)]
        # denominator groups first (tiny mms) so the reciprocal overlaps the
        # attn@V matmuls; all PSUM-group mms stay chunk-serialized per bank
        for b in range(BL):
            for qt in range(QT):
                i = b * QT + qt
                for kt in range(KT):
                    ex = expT[:, kt * W + b * NQ + qt * P:kt * W + b * NQ + (qt + 1) * P]
                    nc.tensor.matmul(
                        den[:, i:i + 1], ex, ones1[:],
                        start=(kt == 0), stop=(kt == KT - 1))
        for i in range(BL * QT):
            nc.vector.reciprocal(rcols[i][:], den[:, i:i + 1])
        for b in range(BL):
            for qt in range(QT):
                i = b * QT + qt
                for kt in range(KT):
                    ex = expT[:, kt * W + b * NQ + qt * P:kt * W + b * NQ + (qt + 1) * P]
                    nc.tensor.matmul(
                        avt[:, i * D:(i + 1) * D],
                        ex, vb[:, (b * KT + kt) * D:(b * KT + kt + 1) * D],
                        start=(kt == 0), stop=(kt == KT - 1))
        for b in range(BL):
            for qt in range(QT):
                i = b * QT + qt
                rc = rcols[i][:]
                av = avt[:, i * D:(i + 1) * D]
                if i % 2 == 0:
                    nc.scalar.activation(out_sbs[i][:], av, AF.Copy,
                                         bias=0.0, scale=rc)
                else:
                    nc.vector.tensor_scalar_mul(out_sbs[i][:], av, rc)
                eng = (nc.sync, nc.gpsimd, nc.scalar, nc.sync)[i]
                eng.dma_start(out_d[b, qt * P:(qt + 1) * P, :], out_sbs[i][:])

    nc.compile()
    return nc


def _host_tables(b1: np.ndarray, w2: np.ndarray):
    tbl = np.zeros((P, TB_N), np.float32)
    tbl[:, TB_HPI] = math.pi / 2.0
    for dt in range(DC):
        w2c = w2[dt * P:(dt + 1) * P]
        tbl[:, TB_WB1 + dt] = W0 * b1[dt * P:(dt + 1) * P]
        tbl[:, TB_WB1H + dt] = W0 * b1[dt * P:(dt + 1) * P] + math.pi / 2.0
        for mi in range(M_SINES):
            tbl[:, TB_W2C + mi * DC + dt] = MCOEF[mi] * w2c
        tbl[:, TB_NW2C2 + dt] = -MCOEF[2] * w2c
        tbl[:, TB_W2C28 + dt] = 0.125 * MCOEF[2] * w2c
    return tbl


_NC_CACHE = {}


def _get_nc():
    if "nc" not in _NC_CACHE:
        _NC_CACHE["nc"] = build_kernel()
    return _NC_CACHE["nc"]


def _make_in_maps(inputs):
    keys = np.ascontiguousarray(np.asarray(inputs["keys"], np.float32).astype(ml_dtypes.bfloat16))
    queries = np.ascontiguousarray(np.asarray(inputs["queries"], np.float32).astype(ml_dtypes.bfloat16))
    values = np.ascontiguousarray(np.asarray(inputs["values"], np.float32).astype(ml_dtypes.bfloat16))
    Wk = np.asarray(inputs["Wk"], np.float32)
    Wq = np.asarray(inputs["Wq"], np.float32)
    b1 = np.asarray(inputs["b1"], np.float64)
    w2 = np.asarray(inputs["w2"], np.float64)

    def wprep(Wm):
        # wqk[p, (ec, dt)*P + j] = Wm[ec*P + p, dt*P + j]
        r = Wm.reshape(EC, P, DC, P).transpose(1, 0, 2, 3).reshape(P, EC * DC * P)
        return np.ascontiguousarray(r.astype(ml_dtypes.bfloat16))

    wq = wprep(Wq)
    wk = wprep(Wk)
    tbl = _host_tables(b1, w2)

    in_maps = []
    for c in range(NCORES):
        sl = slice(c * BL, (c + 1) * BL)
        qk = np.ascontiguousarray(
            np.concatenate([queries[sl], keys[sl]], axis=0).reshape(2 * BL * NQ, D))
        in_maps.append({
            "qk": qk, "values": values[sl],
            "wq": wq, "wk": wk, "tbl": tbl,
        })
    return in_maps


def _run(inputs, trace=False, trace_kwargs=None):
    nc = _get_nc()
    in_maps = _make_in_maps(inputs)
    kwargs = {}
    if trace:
        kwargs = dict(trace=True, trace_cores=[0], trace_kwargs=trace_kwargs or {})
    res = run_bass_kernel_spmd(nc, in_maps, core_ids=list(range(NCORES)), **kwargs)
    out = np.concatenate([res.results[c]["out"] for c in range(NCORES)], axis=0)
    return out, res


def kernel(**inputs) -> np.ndarray:
    out, _ = _run(inputs, trace=False)
    return out
